# revision 2
# baseline (speedup 1.0000x reference)
"""Multi-head attention Bass/Tile kernel for Trainium2, SPMD over 8 NeuronCores.

Sharding: core c handles batch b = c//2 and query rows [qh*1024, qh*1024+1024)
with qh = c%2. Each core computes K/V for its whole batch (duplicated across
the core pair — avoids any cross-core collective), Q for its query half,
full 16-head attention for its queries, and the output projection for its
rows. Host side only slices/transposes/rolls inputs and concatenates the
disjoint output row blocks.

Key layout tricks:
  - keys live on the PSUM partition axis (scores are computed transposed,
    S^T[k, q]), so the attention-mask bias is a per-partition bias fused
    into the ACT exp, and the av matmul (contract over k) needs no on-chip
    transposes;
  - softmax denominators come from a ones-column appended to V (row 64 of
    the z accumulator);
  - no max-subtraction: scaled scores are O(+-10), exp fits fp16/fp32
    comfortably; masked keys get a -1e4 bias so exp underflows to 0.
All matmuls run with fp16 operands (1 cycle/row on the PE, vs 4 for fp32)
accumulating in fp32 PSUM. fp16 keeps 10 mantissa bits and all tensors here
are O(1)-O(300), well within range. Reciprocals are scaled by 256 before
the fp16 round-trip so they stay in the fp16 normal range.
"""
import sys

import numpy as np

sys.path.insert(0, "/opt/trn_rl_repo")

import concourse.bacc as bacc
import concourse.mybir as mybir
import concourse.tile as tile
from concourse import bass_utils
from concourse._compat import get_trn_type
from concourse.bass import ts

P = 128
S = 2048
D = 1024
HE = 1024
NH = 16
NHP = 8
Q = 1024
KT = S // P       # 16 key tiles
DT = D // P       # 8 contraction tiles over d_model
SCALE = 0.125     # 1/sqrt(64)
F32 = mybir.dt.float32
F16 = mybir.dt.float16
I32 = mybir.dt.int32
MUL = mybir.AluOpType.mult
ADD = mybir.AluOpType.add
Exp = mybir.ActivationFunctionType.Exp

N_CORES = 8


def build_nc(stage=4, loop_n=None, sc512=False, vpre=True, ebufs=4, striped=True, vstripe=True, zero_bias=False):
    nc = bacc.Bacc(get_trn_type() or "TRN2", target_bir_lowering=False, debug=False)

    xT = nc.dram_tensor("xT", [D, S], F32, kind="ExternalInput")
    wqT = nc.dram_tensor("wqT", [D, HE], F32, kind="ExternalInput")
    wkT = nc.dram_tensor("wkT", [D, HE], F32, kind="ExternalInput")
    wvT = nc.dram_tensor("wvT", [D, HE], F32, kind="ExternalInput")
    woT = nc.dram_tensor("woT", [HE, D], F32, kind="ExternalInput")
    mask = nc.dram_tensor("mask", [S], I32, kind="ExternalInput")
    bq = nc.dram_tensor("bq", [HE], F32, kind="ExternalInput")
    bk = nc.dram_tensor("bk", [HE], F32, kind="ExternalInput")
    bv = nc.dram_tensor("bv", [HE], F32, kind="ExternalInput")
    bo = nc.dram_tensor("bo", [D], F32, kind="ExternalInput")
    out = nc.dram_tensor("out", [Q, D], F32, kind="ExternalOutput")

    xT_t = xT.rearrange("(dt p) s -> p dt s", p=P)          # [128, 8, 2048]
    wqT_t = wqT.rearrange("(dt p) he -> p dt he", p=P)
    wkT_t = wkT.rearrange("(dt p) he -> p dt he", p=P)
    wvT_t = wvT.rearrange("(dt p) he -> p dt he", p=P)
    woT_t = woT.rearrange("(hp p) d -> p hp d", p=P)        # [128, 8, 1024]
    mask_t = mask.rearrange("(t p) -> p t", p=P)            # [128, 16]
    bq_t = bq.rearrange("(t p) -> p t", p=P)                # [128, 8]
    bk_t = bk.rearrange("(t p) -> p t", p=P)
    out_t = out.rearrange("(qt p) d -> p qt d", p=P)        # [128, 8, 1024]

    import contextlib

    with tile.TileContext(nc) as tc:
        with (
            tc.For_i(0, loop_n, 1) if loop_n else contextlib.nullcontext(),
            tc.tile_pool(name="const", bufs=1) as cpool,
            tc.tile_pool(name="persist", bufs=1) as big,
            tc.tile_pool(name="psum", bufs=1, space="PSUM") as pps,
        ):
            # ---- constants / small tiles
            mask_i = cpool.tile([P, KT], I32)
            nc.sync.dma_start(mask_i[:], mask_t)
            mask_f = cpool.tile([P, KT], F32)
            nc.vector.tensor_copy(mask_f[:], mask_i[:])
            maskb = cpool.tile([P, KT], F32)
            # (m - 1) * 10000 -> 0 for keep, -1e4 for masked
            nc.vector.tensor_scalar(maskb[:], mask_f[:], -1.0, 10000.0, ADD, MUL)
            ones = cpool.tile([P, P], F16)
            nc.vector.memset(ones[:], 1.0)
            bq_sb = cpool.tile([P, DT], F32)
            nc.sync.dma_start(bq_sb[:], bq_t)
            bk_sb = cpool.tile([P, DT], F32)
            nc.sync.dma_start(bk_sb[:], bk_t)
            if not zero_bias:
                bv_sb = cpool.tile([1, HE], F16)
                nc.gpsimd.dma_start(bv_sb[:], bv[None, :])
                bo_sb = cpool.tile([1, D], F16)
                nc.gpsimd.dma_start(bo_sb[:], bo[None, :])

            # ---- persistent fp16 operand tiles: HWDGE f32 loads + DVE casts
            x16 = big.tile([P, DT, S], F16)                  # all of x^T
            with tc.tile_pool(name="stg", bufs=2) as stgpool:
                for dt in range(DT):
                    stg = stgpool.tile([P, S], F32, tag="stg")
                    nc.sync.dma_start(stg[:], xT_t[:, dt, :])
                    nc.vector.tensor_copy(x16[:, dt, :], stg[:])
            kT16 = big.tile([P, NHP, S], F16)                # K^T [he, k]
            v16 = big.tile([P, KT, NH * 65], F16)            # V rows + ones col
            mh16 = big.tile([P, NHP, Q], F16)                # mh^T [he, q]

            vv = v16[:].rearrange("p t (h e) -> p t h e", e=65)
            nc.vector.memset(vv[:, :, :, 64:65], 1.0)

            def kproj_chunk(het, chunk):
                ps = pps.tile([P, Q] if not sc512 else [P, 512], F32, tag="sc", name="psk", bufs=2)
                pss = ps[:, 0:512]
                for dt in range(DT):
                    nc.tensor.matmul(
                        pss, wk16[:, dt, ts(het, P)],
                        x16[:, dt, ts(chunk, 512)],
                        start=(dt == 0), stop=(dt == DT - 1))
                nc.vector.tensor_tensor(
                    kT16[:, het, ts(chunk, 512)], pss,
                    bk_sb[:, het:het + 1].to_broadcast((P, 512)), ADD)

            def kproj(het):
                """K^T rows for head pair het -> kT16[:, het, :]."""
                for chunk in range(4):
                    kproj_chunk(het, chunk)

            def vproj(kt):
                """V rows for key tile kt (all heads) + bias + ones col."""
                for nch in range(2):
                    ps = pps.tile([P, Q] if not sc512 else [P, 512], F32, tag="sc", name="psv", bufs=2)
                    pss = ps[:, 0:512]
                    for dt in range(DT):
                        nc.tensor.matmul(
                            pss, x16[:, dt, ts(kt, P)],
                            wv16[:, dt, ts(nch, 512)],
                            start=(dt == 0),
                            stop=(zero_bias and dt == DT - 1))
                    if not zero_bias:
                        nc.tensor.matmul(
                            pss, ones[0:1, 0:P], bv_sb[0:1, ts(nch, 512)],
                            start=False, stop=True)
                    dst = vv[:, kt, nch * 8:(nch + 1) * 8, 0:64]
                    nc.vector.tensor_copy(
                        dst, pss.rearrange("p (h e) -> p h e", e=64))

            def qproj_chunk(hp, qT, qch):
                ps = pps.tile([P, Q] if not sc512 else [P, 512], F32, tag="sc", name="psq", bufs=2)
                pss = ps[:, 0:512]
                for dt in range(DT):
                    nc.tensor.matmul(
                        pss, wq16[:, dt, ts(hp, P)],
                        x16[:, dt, ts(qch, 512)],
                        start=(dt == 0), stop=(dt == DT - 1))
                nc.vector.tensor_tensor(
                    qT[:, ts(qch, 512)], pss,
                    bq_sb[:, hp:hp + 1].to_broadcast((P, 512)), ADD)

            def qproj(hp, qT):
                for qch in range(2):
                    qproj_chunk(hp, qT, qch)

            with (
                tc.tile_pool(name="wts", bufs=1) as wpool,
                tc.tile_pool(name="qt", bufs=2) as qpool,
                tc.tile_pool(name="exp", bufs=ebufs) as epool,
                tc.tile_pool(name="fin", bufs=1) as fpool,
            ):
                wk16 = wpool.tile([P, DT, HE], F16)
                wv16 = wpool.tile([P, DT, HE], F16)
                wq16 = wpool.tile([P, DT, HE], F16)
                with tc.tile_pool(name="wstg", bufs=2) as wstg:
                    for w16, wt in ((wk16, wkT_t), (wv16, wvT_t),
                                    (wq16, wqT_t)):
                        for dt in range(DT):
                            stg = wstg.tile([P, HE], F32, tag="wstg")
                            nc.sync.dma_start(stg[:], wt[:, dt, :])
                            nc.vector.tensor_copy(w16[:, dt, :], stg[:])

                if stage >= 2:
                    kproj(0)
                    qT_tiles = {0: qpool.tile([P, Q], F16, tag="qTn", name="qT0")}
                    qproj(0, qT_tiles[0])
                if stage == 2:
                    for hp in range(1, NHP):
                        kproj(hp)
                        qT_tiles[hp % 2] = qpool.tile([P, Q], F16, name="qTn")
                        qproj(hp, qT_tiles[hp % 2])
                    for kt in range(KT):
                        vproj(kt)
                def scores_exp(hp, qT, kt):
                    """Emit the 4 score matmuls (hh interleaved so adjacent
                    matmuls sit in different PE row groups and overlap) and
                    the exps for key tile kt. Returns the exp tiles."""
                    if sc512:
                        ets = [epool.tile([P, Q], F16, name=f"et{hh}")
                               for hh in range(2)]
                        for qch in range(2):
                            scs = []
                            for hh in range(2):
                                sc = pps.tile([P, 512], F32,
                                              tag=("sc" if hh == 0 else "sc2"),
                                              name=f"sc{hh}", bufs=2)
                                scs.append(sc)
                            for hh in range(2):
                                nc.tensor.matmul(
                                    scs[hh][:],
                                    kT16[hh * 64:(hh + 1) * 64, hp, ts(kt, P)],
                                    qT[hh * 64:(hh + 1) * 64, ts(qch, 512)],
                                    start=True, stop=True)
                            for hh in range(2):
                                nc.scalar.activation(
                                    ets[hh][:, ts(qch, 512)], scs[hh][:], Exp,
                                    bias=maskb[:, kt:kt + 1], scale=SCALE)
                        return ets
                    scs = []
                    for hh in range(2):
                        scs.append(pps.tile([P, Q], F32, tag="sc",
                                            name=f"sc{hh}", bufs=2))
                    for qch in range(2):
                        for hh in range(2):
                            nc.tensor.matmul(
                                scs[hh][:, ts(qch, 512)],
                                kT16[hh * 64:(hh + 1) * 64, hp, ts(kt, P)],
                                qT[hh * 64:(hh + 1) * 64, ts(qch, 512)],
                                start=True, stop=True)
                    ets = []
                    for hh in range(2):
                        et = epool.tile([P, Q], F16, name=f"et{hh}")
                        nc.scalar.activation(
                            et[:], scs[hh][:], Exp,
                            bias=maskb[:, kt:kt + 1], scale=SCALE)
                        ets.append(et)
                    return ets

                def av(hp, zt, kt, ets):
                    for hh in range(2):
                        h = hp * 2 + hh
                        for qch in range(2):
                            nc.tensor.matmul(
                                zt[hh * 2 + qch][:],
                                vv[:, kt, h, 0:65],
                                ets[hh][:, ts(qch, 512)],
                                start=(kt == 0), stop=(kt == KT - 1))

                if stage >= 3 and vpre:
                    for kt in range(4 if vstripe else KT):
                        vproj(kt)
                for hp in range(NHP if stage >= 3 else 0):
                    qT = qT_tiles.pop(hp)

                    zt = [pps.tile([65, 512], F32, tag=f"z{i}", name=f"z{i}")
                          for i in range(4)]
                    ets = scores_exp(hp, qT, 0)
                    for kt in range(KT):
                        if hp == 0 and not vpre:
                            vproj(kt)
                        if hp == 0 and vpre and vstripe and kt + 4 < KT:
                            vproj(kt + 4)
                        nxt = (scores_exp(hp, qT, kt + 1)
                               if kt + 1 < KT else None)
                        av(hp, zt, kt, ets)
                        ets = nxt
                        if striped and hp + 1 < NHP:
                            if kt in (4, 7, 10, 13):
                                kproj_chunk(hp + 1, (kt - 4) // 3)
                            elif kt == 5:
                                qT_tiles[hp + 1] = qpool.tile(
                                    [P, Q], F16, name="qTn")
                                qproj_chunk(hp + 1, qT_tiles[hp + 1], 0)
                            elif kt == 11:
                                qproj_chunk(hp + 1, qT_tiles[hp + 1], 1)
                    # next head-pair's projections run while ACT drains the
                    # last exps and DVE runs the normalize below
                    if not striped and hp + 1 < NHP:
                        kproj(hp + 1)
                        qT_tiles[hp + 1] = qpool.tile([P, Q], F16, name="qTn")
                        qproj(hp + 1, qT_tiles[hp + 1])

                    # --- normalize z and write mh^T (denominator in row 64)
                    # copy z to SBUF immediately to release the PSUM banks
                    zc = fpool.tile([65, 4, 512], F32, tag="zc")
                    for i in range(4):
                        nc.vector.tensor_copy(zc[:, i, :], zt[i][:])
                    nc.vector.reciprocal(zc[64:65, :, :], zc[64:65, :, :])
                    r16 = fpool.tile([65, 4, 512], F16, tag="r16")
                    with nc.allow_low_precision(reason="softmax recip f16"):
                        nc.vector.tensor_scalar_mul(
                            r16[64:65, :, :], zc[64:65, :, :], 256.0)
                    for hh in range(2):
                        for qch in range(2):
                            i = hh * 2 + qch
                            rep = pps.tile([64, 512], F32, tag=f"z{i}",
                                           name="rep")
                            nc.tensor.matmul(
                                rep[:], ones[64:65, 0:64],
                                r16[64:65, i, :], start=True, stop=True)
                            if hh == 0:
                                nc.vector.scalar_tensor_tensor(
                                    mh16[0:64, hp, ts(qch, 512)], zc[0:64, i, :],
                                    1.0 / 256.0, rep[:], MUL, MUL)
                            else:
                                tmp = fpool.tile([64, 512], F16, tag="tmp")
                                nc.vector.scalar_tensor_tensor(
                                    tmp[:], zc[0:64, i, :], 1.0 / 256.0, rep[:],
                                    MUL, MUL)
                                nc.sync.dma_start(
                                    mh16[64:128, hp, ts(qch, 512)], tmp[:])

            # ================= output projection ====================
            with (
                tc.tile_pool(name="wo", bufs=1) as wopool,
                tc.tile_pool(name="ost", bufs=3) as opool,
            ):
                wo16 = wopool.tile([P, NHP, D], F16)
                with tc.tile_pool(name="wostg", bufs=2) as wostg:
                    for dt in range(DT):
                        stg = wostg.tile([P, D], F32, tag="wostg")
                        nc.sync.dma_start(stg[:], woT_t[:, dt, :])
                        nc.vector.tensor_copy(wo16[:, dt, :], stg[:])
                for nch in range(2):
                    if stage < 4:
                        break
                    for qt in range(8):
                        ps = pps.tile([P, Q] if not sc512 else [P, 512], F32, tag="sc", name="pso", bufs=2)
                        pss = ps[:, 0:512]
                        for hp in range(NHP):
                            nc.tensor.matmul(
                                pss, mh16[:, hp, ts(qt, P)],
                                wo16[:, hp, ts(nch, 512)],
                                start=(hp == 0),
                                stop=(zero_bias and hp == NHP - 1))
                        if not zero_bias:
                            nc.tensor.matmul(
                                pss, ones[0:1, 0:P], bo_sb[0:1, ts(nch, 512)],
                                start=False, stop=True)
                        ot = opool.tile([P, 512], F32)
                        nc.vector.tensor_copy(ot[:], pss)
                        nc.sync.dma_start(out_t[:, qt, ts(nch, 512)], ot[:])

    nc.compile()
    return nc


_NC_CACHE = {}


def _get_nc(zero_bias=False):
    if zero_bias not in _NC_CACHE:
        _NC_CACHE[zero_bias] = build_nc(zero_bias=zero_bias)
    return _NC_CACHE[zero_bias]


def make_in_maps(x, attention_mask, Wq, bq, Wk, bk, Wv, bv, Wo, bo):
    x = np.ascontiguousarray(np.asarray(x, dtype=np.float32))
    attention_mask = np.asarray(attention_mask, dtype=np.int32)
    wqT = np.ascontiguousarray(np.asarray(Wq).transpose(2, 0, 1).reshape(D, HE))
    wkT = np.ascontiguousarray(np.asarray(Wk).transpose(2, 0, 1).reshape(D, HE))
    wvT = np.ascontiguousarray(np.asarray(Wv).transpose(2, 0, 1).reshape(D, HE))
    woT = np.ascontiguousarray(np.asarray(Wo).T.astype(np.float32))
    bqf = np.asarray(bq, dtype=np.float32).reshape(HE)
    bkf = np.asarray(bk, dtype=np.float32).reshape(HE)
    bvf = np.asarray(bv, dtype=np.float32).reshape(HE)
    bof = np.asarray(bo, dtype=np.float32).reshape(D)

    in_maps = []
    for c in range(N_CORES):
        b, qh = c // 2, c % 2
        qoff = qh * Q
        xTc = np.ascontiguousarray(np.roll(x[b].T, -qoff, axis=1))
        mk = np.ascontiguousarray(np.roll(attention_mask[b, 0], -qoff))
        in_maps.append({
            "xT": xTc, "wqT": wqT, "wkT": wkT, "wvT": wvT, "woT": woT,
            "mask": mk, "bq": bqf, "bk": bkf, "bv": bvf, "bo": bof,
        })
    return in_maps


def kernel(x, attention_mask, Wq, bq, Wk, bk, Wv, bv, Wo, bo, trace=False):
    B = np.asarray(x).shape[0]
    in_maps = make_in_maps(x, attention_mask, Wq, bq, Wk, bk, Wv, bv, Wo, bo)
    bqf = in_maps[0]["bq"]
    zb = bool(np.all(in_maps[0]["bq"] == 0) and np.all(in_maps[0]["bk"] == 0)
              and np.all(in_maps[0]["bv"] == 0) and np.all(in_maps[0]["bo"] == 0))
    nc = _get_nc(zero_bias=zb)
    res = bass_utils.run_bass_kernel_spmd(
        nc, in_maps, core_ids=list(range(N_CORES)), trace=trace)
    out = np.empty((B, S, D), dtype=np.float32)
    for c in range(N_CORES):
        b, qh = c // 2, c % 2
        out[b, qh * Q:(qh + 1) * Q, :] = res.results[c]["out"]
    kernel.last_result = res
    return out



# revision 3
# speedup vs baseline: 1.5203x; 1.5203x over previous
"""Multi-head attention Bass/Tile kernel for Trainium2, SPMD over 8 NeuronCores.

Sharding: core c handles batch b = c//2 and query rows [qh*1024, qh*1024+1024)
with qh = c%2. Each core computes K/V for its whole batch (duplicated across
the core pair -- avoids any cross-core collective), Q for its query half,
full 16-head attention for its queries, and the output projection for its
rows. Host side slices/transposes inputs, casts them to fp16, and --- the
big one --- COMPACTS the key axis: keys with attention_mask==0 contribute
exactly 0 after the -1e4 exp bias underflows, so the host gathers only the
unmasked keys (~half for a Bernoulli(0.5) mask) and pads to a multiple of
128. K/V projection, scores, exp and AV all shrink proportionally.

Key layout tricks (from the v1 kernel):
  - keys live on the PSUM partition axis (scores are computed transposed,
    S^T[k, q]), so the attention-mask bias is a per-partition bias fused
    into the ACT exp, and the av matmul (contract over k) needs no on-chip
    transposes;
  - softmax denominators come from a ones-column appended to V (row 64 of
    the z accumulator);
  - no max-subtraction: scaled scores are O(+-10), exp fits fp16/fp32
    comfortably; masked/pad keys get a -1e4 bias so exp underflows to 0.
All matmuls run with fp16 operands (1 cycle/row on the PE vs 4 for fp32)
accumulating in fp32 PSUM. Inputs are cast to fp16 on the host, halving
input DMA and eliminating all on-chip staging casts. Reciprocals are
scaled by 256 before the fp16 round-trip so they stay in the fp16 normal
range.

Schedule: qproj first (only wq+xq need to have landed), so the K/V/O input
DMA streams in under it; kproj(hp+1) is striped into head-pair hp's
attention loop and vproj(kt+3) into hp0's, keeping the PE dense while the
ACT exp runs ahead by one key tile.
"""
import sys

import numpy as np

sys.path.insert(0, "/opt/trn_rl_repo")

import concourse.bacc as bacc
import concourse.mybir as mybir
import concourse.tile as tile
from concourse import bass_utils
from concourse._compat import get_trn_type
from concourse.bass import ts

P = 128
S = 2048
D = 1024
HE = 1024
NH = 16
NHP = 8
Q = 1024
DT = D // P       # 8 contraction tiles over d_model
SCALE = 0.125     # 1/sqrt(64)
F32 = mybir.dt.float32
F16 = mybir.dt.float16
I32 = mybir.dt.int32
MUL = mybir.AluOpType.mult
ADD = mybir.AluOpType.add
Exp = mybir.ActivationFunctionType.Exp

N_CORES = 8
MAX_KTE_FAST = 10   # fast (compacted) build fits SBUF up to this many key tiles


def build_nc(kte=9, loop_n=None, zero_bias=True, ebufs=4):
    """Compacted-key build: SK = kte*128 keys (pad keys masked to -1e4)."""
    SK = kte * P
    SCH = (SK + 511) // 512          # kproj chunk count (chunks of <=512)
    nc = bacc.Bacc(get_trn_type() or "TRN2", target_bir_lowering=False,
                   debug=False)

    xq = nc.dram_tensor("xq", [D, Q], F16, kind="ExternalInput")
    xk = nc.dram_tensor("xk", [D, SK], F16, kind="ExternalInput")
    wqT = nc.dram_tensor("wqT", [D, HE], F16, kind="ExternalInput")
    wkT = nc.dram_tensor("wkT", [D, HE], F16, kind="ExternalInput")
    wvT = nc.dram_tensor("wvT", [D, HE], F16, kind="ExternalInput")
    woT = nc.dram_tensor("woT", [HE, D], F16, kind="ExternalInput")
    mask = nc.dram_tensor("mask", [SK], I32, kind="ExternalInput")
    bq = nc.dram_tensor("bq", [HE], F32, kind="ExternalInput")
    bk = nc.dram_tensor("bk", [HE], F32, kind="ExternalInput")
    bv = nc.dram_tensor("bv", [HE], F32, kind="ExternalInput")
    bo = nc.dram_tensor("bo", [D], F32, kind="ExternalInput")
    out = nc.dram_tensor("out", [Q, D], F32, kind="ExternalOutput")

    xq_t = xq.rearrange("(dt p) q -> p dt q", p=P)
    xk_t = xk.rearrange("(dt p) s -> p dt s", p=P)
    wq_t = wqT.rearrange("(dt p) he -> p dt he", p=P)
    wk_t = wkT.rearrange("(dt p) he -> p dt he", p=P)
    wv_t = wvT.rearrange("(dt p) he -> p dt he", p=P)
    wo_t = woT.rearrange("(hp p) d -> p hp d", p=P)
    mask_t = mask.rearrange("(t p) -> p t", p=P)            # [128, kte]
    bq_t = bq.rearrange("(t p) -> p t", p=P)                # [128, 8]
    bk_t = bk.rearrange("(t p) -> p t", p=P)
    out_t = out.rearrange("(qt p) d -> p qt d", p=P)        # [128, 8, 1024]

    import contextlib

    with tile.TileContext(nc) as tc:
        with (
            tc.For_i(0, loop_n, 1) if loop_n else contextlib.nullcontext(),
            tc.tile_pool(name="const", bufs=1) as cpool,
            tc.tile_pool(name="persist", bufs=1) as big,
            tc.tile_pool(name="psum", bufs=1, space="PSUM") as pps,
        ):
            # ---- constants / small tiles (gpsimd queue: off the main ring)
            mask_i = cpool.tile([P, kte], I32)
            nc.gpsimd.dma_start(mask_i[:], mask_t)
            mask_f = cpool.tile([P, kte], F32)
            nc.vector.tensor_copy(mask_f[:], mask_i[:])
            maskb = cpool.tile([P, kte], F32)
            # (m - 1) * 10000 -> 0 for keep, -1e4 for masked/pad
            nc.vector.tensor_scalar(maskb[:], mask_f[:], -1.0, 10000.0,
                                    ADD, MUL)
            ones = cpool.tile([P, P], F16)
            nc.vector.memset(ones[:], 1.0)
            bq_sb = cpool.tile([P, DT], F32)
            nc.gpsimd.dma_start(bq_sb[:], bq_t)
            bk_sb = cpool.tile([P, DT], F32)
            nc.gpsimd.dma_start(bk_sb[:], bk_t)
            if not zero_bias:
                bv_sb = cpool.tile([1, HE], F16)
                nc.gpsimd.dma_start(bv_sb[:], bv[None, :])
                bo_sb = cpool.tile([1, D], F16)
                nc.gpsimd.dma_start(bo_sb[:], bo[None, :])

            # ---- persistent fp16 tiles, DMA'd directly (host pre-cast)
            wq16 = big.tile([P, DT, HE], F16)
            xq16 = big.tile([P, DT, Q], F16)
            wk16 = big.tile([P, DT, HE], F16)
            xk16 = big.tile([P, DT, SK], F16)
            wv16 = big.tile([P, DT, HE], F16)
            wo16 = big.tile([P, NHP, D], F16)
            kT16 = big.tile([P, NHP, SK], F16)               # K^T [he, k]
            v16 = big.tile([P, kte, NH * 65], F16)           # V rows + ones col
            mh16 = big.tile([P, NHP, Q], F16)                # mh^T [he, q]
            qT16 = big.tile([P, NHP, Q], F16)                # Q^T [he, q]

            # issue order == consumption order: q-side first, then k, v, o
            nc.sync.dma_start(wq16[:], wq_t)
            nc.sync.dma_start(xq16[:], xq_t)
            nc.sync.dma_start(wk16[:], wk_t)
            nc.sync.dma_start(xk16[:], xk_t)
            nc.sync.dma_start(wv16[:], wv_t)
            nc.sync.dma_start(wo16[:], wo_t)

            vv = v16[:].rearrange("p t (h e) -> p t h e", e=65)
            nc.vector.memset(vv[:, :, :, 64:65], 1.0)

            def qproj(hp):
                for qch in range(2):
                    ps = pps.tile([P, Q], F32, tag="sc", name="psq", bufs=2)
                    pss = ps[:, 0:512]
                    for dt in range(DT):
                        nc.tensor.matmul(
                            pss, wq16[:, dt, ts(hp, P)],
                            xq16[:, dt, ts(qch, 512)],
                            start=(dt == 0), stop=(dt == DT - 1))
                    if zero_bias:
                        nc.vector.tensor_copy(
                            qT16[:, hp, ts(qch, 512)], pss)
                    else:
                        nc.vector.tensor_tensor(
                            qT16[:, hp, ts(qch, 512)], pss,
                            bq_sb[:, hp:hp + 1].to_broadcast((P, 512)), ADD)

            def kproj_chunk(hp, ch):
                w = min(512, SK - ch * 512)
                ps = pps.tile([P, Q], F32, tag="sc", name="psk", bufs=2)
                pss = ps[:, 0:w]
                for dt in range(DT):
                    nc.tensor.matmul(
                        pss, wk16[:, dt, ts(hp, P)],
                        xk16[:, dt, ch * 512:ch * 512 + w],
                        start=(dt == 0), stop=(dt == DT - 1))
                if zero_bias:
                    nc.vector.tensor_copy(
                        kT16[:, hp, ch * 512:ch * 512 + w], pss)
                else:
                    nc.vector.tensor_tensor(
                        kT16[:, hp, ch * 512:ch * 512 + w], pss,
                        bk_sb[:, hp:hp + 1].to_broadcast((P, w)), ADD)

            def kproj(hp):
                for ch in range(SCH):
                    kproj_chunk(hp, ch)

            def vproj(kt):
                for nch in range(2):
                    ps = pps.tile([P, Q], F32, tag="sc", name="psv", bufs=2)
                    pss = ps[:, 0:512]
                    for dt in range(DT):
                        nc.tensor.matmul(
                            pss, xk16[:, dt, ts(kt, P)],
                            wv16[:, dt, ts(nch, 512)],
                            start=(dt == 0),
                            stop=(zero_bias and dt == DT - 1))
                    if not zero_bias:
                        nc.tensor.matmul(
                            pss, ones[0:1, 0:P], bv_sb[0:1, ts(nch, 512)],
                            start=False, stop=True)
                    dst = vv[:, kt, nch * 8:(nch + 1) * 8, 0:64]
                    nc.vector.tensor_copy(
                        dst, pss.rearrange("p (h e) -> p h e", e=64))

            def scores_exp(hp, kt):
                """4 score matmuls for key tile kt (hh at base partitions
                0/64 -> distinct PE row groups, may overlap) + fused
                mask-bias exp on ACT. Returns the 2 exp tiles."""
                scs = []
                for hh in range(2):
                    scs.append(pps.tile([P, Q], F32, tag="sc",
                                        name=f"sc{hh}", bufs=2))
                for qch in range(2):
                    for hh in range(2):
                        nc.tensor.matmul(
                            scs[hh][:, ts(qch, 512)],
                            kT16[hh * 64:(hh + 1) * 64, hp, ts(kt, P)],
                            qT16[hh * 64:(hh + 1) * 64, hp, ts(qch, 512)],
                            start=True, stop=True)
                ets = []
                for hh in range(2):
                    et = epool.tile([P, Q], F16, name=f"et{hh}")
                    nc.scalar.activation(
                        et[:], scs[hh][:], Exp,
                        bias=maskb[:, kt:kt + 1], scale=SCALE)
                    ets.append(et)
                return ets

            def av(hp, zt, kt, ets):
                for hh in range(2):
                    h = hp * 2 + hh
                    for qch in range(2):
                        nc.tensor.matmul(
                            zt[hh * 2 + qch][:],
                            vv[:, kt, h, 0:65],
                            ets[hh][:, ts(qch, 512)],
                            start=(kt == 0), stop=(kt == kte - 1))

            with (
                tc.tile_pool(name="exp", bufs=ebufs) as epool,
                tc.tile_pool(name="fin", bufs=1) as fpool,
            ):
                # ---- projection prologue: q-side only (k/v DMAs stream
                # in underneath), then kproj(0) and the first v tiles.
                for hp in range(NHP):
                    qproj(hp)
                kproj(0)
                VP0 = min(3, kte)
                striped = kte >= 4
                for kt in range(VP0 if striped else kte):
                    vproj(kt)
                if not striped:
                    for hp in range(1, NHP):
                        kproj(hp)

                # stripe schedule: within head-pair hp's kt loop, run
                # vproj(kt+VP0) (hp==0) and the next hp's kproj chunks.
                def stripe_ops(hp, kt):
                    if not striped:
                        return
                    if hp == 0 and kt + VP0 < kte:
                        vproj(kt + VP0)
                    if hp + 1 < NHP:
                        # spread SCH kproj chunks over kt slots 1..kte-2
                        nops = SCH
                        for i in range(nops):
                            slot = 1 + ((i * max(1, (kte - 2))) // nops)
                            if kt == slot:
                                kproj_chunk(hp + 1, i)

                for hp in range(NHP):
                    zt = [pps.tile([65, 512], F32, tag=f"z{i}", name=f"z{i}")
                          for i in range(4)]
                    ets = scores_exp(hp, 0)
                    for kt in range(kte):
                        nxt = (scores_exp(hp, kt + 1)
                               if kt + 1 < kte else None)
                        av(hp, zt, kt, ets)
                        ets = nxt
                        stripe_ops(hp, kt)

                    # --- normalize z and write mh^T (denominator row 64)
                    zc = fpool.tile([65, 4, 512], F32, tag="zc")
                    for i in range(4):
                        nc.vector.tensor_copy(zc[:, i, :], zt[i][:])
                    nc.vector.reciprocal(zc[64:65, :, :], zc[64:65, :, :])
                    r16 = fpool.tile([65, 4, 512], F16, tag="r16")
                    with nc.allow_low_precision(reason="softmax recip f16"):
                        nc.vector.tensor_scalar_mul(
                            r16[64:65, :, :], zc[64:65, :, :], 256.0)
                    for hh in range(2):
                        for qch in range(2):
                            i = hh * 2 + qch
                            rep = pps.tile([64, 512], F32, tag=f"z{i}",
                                           name="rep")
                            nc.tensor.matmul(
                                rep[:], ones[64:65, 0:64],
                                r16[64:65, i, :], start=True, stop=True)
                            if hh == 0:
                                nc.vector.scalar_tensor_tensor(
                                    mh16[0:64, hp, ts(qch, 512)],
                                    zc[0:64, i, :],
                                    1.0 / 256.0, rep[:], MUL, MUL)
                            else:
                                tmp = fpool.tile([64, 512], F16, tag="tmp")
                                nc.vector.scalar_tensor_tensor(
                                    tmp[:], zc[0:64, i, :], 1.0 / 256.0,
                                    rep[:], MUL, MUL)
                                nc.sync.dma_start(
                                    mh16[64:128, hp, ts(qch, 512)], tmp[:])

            # ================= output projection ====================
            with tc.tile_pool(name="ost", bufs=3) as opool:
                for nch in range(2):
                    for qt in range(8):
                        ps = pps.tile([P, Q], F32, tag="sc", name="pso",
                                      bufs=2)
                        pss = ps[:, 0:512]
                        for hp in range(NHP):
                            nc.tensor.matmul(
                                pss, mh16[:, hp, ts(qt, P)],
                                wo16[:, hp, ts(nch, 512)],
                                start=(hp == 0),
                                stop=(zero_bias and hp == NHP - 1))
                        if not zero_bias:
                            nc.tensor.matmul(
                                pss, ones[0:1, 0:P], bo_sb[0:1, ts(nch, 512)],
                                start=False, stop=True)
                        ot = opool.tile([P, 512], F32)
                        nc.vector.tensor_copy(ot[:], pss)
                        nc.sync.dma_start(out_t[:, qt, ts(nch, 512)], ot[:])

    nc.compile()
    return nc


def make_in_maps(x, attention_mask, Wq, bq, Wk, bk, Wv, bv, Wo, bo):
    """Host preprocessing for the fast build. Returns (in_maps, kte)."""
    x = np.asarray(x, dtype=np.float32)
    attention_mask = np.asarray(attention_mask, dtype=np.int32)
    B = x.shape[0]
    counts = [int(attention_mask[b, 0].sum()) for b in range(B)]
    kte = max(1, -(-max(counts) // P))
    if kte > MAX_KTE_FAST:
        return None, kte
    SK = kte * P

    wqT = np.asarray(Wq).transpose(2, 0, 1).reshape(D, HE).astype(np.float16)
    wkT = np.asarray(Wk).transpose(2, 0, 1).reshape(D, HE).astype(np.float16)
    wvT = np.asarray(Wv).transpose(2, 0, 1).reshape(D, HE).astype(np.float16)
    woT = np.ascontiguousarray(np.asarray(Wo).T).astype(np.float16)
    bqf = np.asarray(bq, dtype=np.float32).reshape(HE)
    bkf = np.asarray(bk, dtype=np.float32).reshape(HE)
    bvf = np.asarray(bv, dtype=np.float32).reshape(HE)
    bof = np.asarray(bo, dtype=np.float32).reshape(D)

    in_maps = []
    for c in range(N_CORES):
        b, qh = c // 2, c % 2
        if qh == 0:
            idx = np.nonzero(attention_mask[b, 0])[0]
            xT = x[b].T.astype(np.float16)
            xkc = np.zeros((D, SK), dtype=np.float16)
            xkc[:, :len(idx)] = xT[:, idx]
            mk = np.zeros(SK, dtype=np.int32)
            mk[:len(idx)] = 1
        xqc = np.ascontiguousarray(xT[:, qh * Q:(qh + 1) * Q])
        in_maps.append({
            "xq": xqc, "xk": xkc, "wqT": wqT, "wkT": wkT, "wvT": wvT,
            "woT": woT, "mask": mk, "bq": bqf, "bk": bkf, "bv": bvf,
            "bo": bof,
        })
    return in_maps, kte


_NC_CACHE = {}


def _get_nc(kte, zero_bias):
    key = (kte, zero_bias)
    if key not in _NC_CACHE:
        _NC_CACHE[key] = build_nc(kte=kte, zero_bias=zero_bias)
    return _NC_CACHE[key]


def kernel(x, attention_mask, Wq, bq, Wk, bk, Wv, bv, Wo, bo, trace=False):
    B = np.asarray(x).shape[0]
    args = (x, attention_mask, Wq, bq, Wk, bk, Wv, bv, Wo, bo)
    in_maps, kte = make_in_maps(*args)
    if in_maps is None:
        return _kernel_dense(*args)
    zb = bool(np.all(in_maps[0]["bq"] == 0) and np.all(in_maps[0]["bk"] == 0)
              and np.all(in_maps[0]["bv"] == 0) and np.all(in_maps[0]["bo"] == 0))
    nc = _get_nc(kte, zb)
    res = bass_utils.run_bass_kernel_spmd(
        nc, in_maps, core_ids=list(range(N_CORES)), trace=trace)
    out = np.empty((B, S, D), dtype=np.float32)
    for c in range(N_CORES):
        b, qh = c // 2, c % 2
        out[b, qh * Q:(qh + 1) * Q, :] = res.results[c]["out"]
    kernel.last_result = res
    return out


# ======================================================================
# Dense fallback (mask too dense to compact): the v1 kernel, unchanged.
# ======================================================================
KT = S // P


def build_nc_dense(stage=4, loop_n=None, sc512=False, vpre=True, ebufs=4,
                   striped=True, vstripe=True, zero_bias=False):
    nc = bacc.Bacc(get_trn_type() or "TRN2", target_bir_lowering=False,
                   debug=False)

    xT = nc.dram_tensor("xT", [D, S], F32, kind="ExternalInput")
    wqT = nc.dram_tensor("wqT", [D, HE], F32, kind="ExternalInput")
    wkT = nc.dram_tensor("wkT", [D, HE], F32, kind="ExternalInput")
    wvT = nc.dram_tensor("wvT", [D, HE], F32, kind="ExternalInput")
    woT = nc.dram_tensor("woT", [HE, D], F32, kind="ExternalInput")
    mask = nc.dram_tensor("mask", [S], I32, kind="ExternalInput")
    bq = nc.dram_tensor("bq", [HE], F32, kind="ExternalInput")
    bk = nc.dram_tensor("bk", [HE], F32, kind="ExternalInput")
    bv = nc.dram_tensor("bv", [HE], F32, kind="ExternalInput")
    bo = nc.dram_tensor("bo", [D], F32, kind="ExternalInput")
    out = nc.dram_tensor("out", [Q, D], F32, kind="ExternalOutput")

    xT_t = xT.rearrange("(dt p) s -> p dt s", p=P)          # [128, 8, 2048]
    wqT_t = wqT.rearrange("(dt p) he -> p dt he", p=P)
    wkT_t = wkT.rearrange("(dt p) he -> p dt he", p=P)
    wvT_t = wvT.rearrange("(dt p) he -> p dt he", p=P)
    woT_t = woT.rearrange("(hp p) d -> p hp d", p=P)        # [128, 8, 1024]
    mask_t = mask.rearrange("(t p) -> p t", p=P)            # [128, 16]
    bq_t = bq.rearrange("(t p) -> p t", p=P)                # [128, 8]
    bk_t = bk.rearrange("(t p) -> p t", p=P)
    out_t = out.rearrange("(qt p) d -> p qt d", p=P)        # [128, 8, 1024]

    import contextlib

    with tile.TileContext(nc) as tc:
        with (
            tc.For_i(0, loop_n, 1) if loop_n else contextlib.nullcontext(),
            tc.tile_pool(name="const", bufs=1) as cpool,
            tc.tile_pool(name="persist", bufs=1) as big,
            tc.tile_pool(name="psum", bufs=1, space="PSUM") as pps,
        ):
            # ---- constants / small tiles
            mask_i = cpool.tile([P, KT], I32)
            nc.sync.dma_start(mask_i[:], mask_t)
            mask_f = cpool.tile([P, KT], F32)
            nc.vector.tensor_copy(mask_f[:], mask_i[:])
            maskb = cpool.tile([P, KT], F32)
            # (m - 1) * 10000 -> 0 for keep, -1e4 for masked
            nc.vector.tensor_scalar(maskb[:], mask_f[:], -1.0, 10000.0, ADD, MUL)
            ones = cpool.tile([P, P], F16)
            nc.vector.memset(ones[:], 1.0)
            bq_sb = cpool.tile([P, DT], F32)
            nc.sync.dma_start(bq_sb[:], bq_t)
            bk_sb = cpool.tile([P, DT], F32)
            nc.sync.dma_start(bk_sb[:], bk_t)
            if not zero_bias:
                bv_sb = cpool.tile([1, HE], F16)
                nc.gpsimd.dma_start(bv_sb[:], bv[None, :])
                bo_sb = cpool.tile([1, D], F16)
                nc.gpsimd.dma_start(bo_sb[:], bo[None, :])

            # ---- persistent fp16 operand tiles: HWDGE f32 loads + DVE casts
            x16 = big.tile([P, DT, S], F16)                  # all of x^T
            with tc.tile_pool(name="stg", bufs=2) as stgpool:
                for dt in range(DT):
                    stg = stgpool.tile([P, S], F32, tag="stg")
                    nc.sync.dma_start(stg[:], xT_t[:, dt, :])
                    nc.vector.tensor_copy(x16[:, dt, :], stg[:])
            kT16 = big.tile([P, NHP, S], F16)                # K^T [he, k]
            v16 = big.tile([P, KT, NH * 65], F16)            # V rows + ones col
            mh16 = big.tile([P, NHP, Q], F16)                # mh^T [he, q]

            vv = v16[:].rearrange("p t (h e) -> p t h e", e=65)
            nc.vector.memset(vv[:, :, :, 64:65], 1.0)

            def kproj_chunk(het, chunk):
                ps = pps.tile([P, Q] if not sc512 else [P, 512], F32, tag="sc", name="psk", bufs=2)
                pss = ps[:, 0:512]
                for dt in range(DT):
                    nc.tensor.matmul(
                        pss, wk16[:, dt, ts(het, P)],
                        x16[:, dt, ts(chunk, 512)],
                        start=(dt == 0), stop=(dt == DT - 1))
                nc.vector.tensor_tensor(
                    kT16[:, het, ts(chunk, 512)], pss,
                    bk_sb[:, het:het + 1].to_broadcast((P, 512)), ADD)

            def kproj(het):
                """K^T rows for head pair het -> kT16[:, het, :]."""
                for chunk in range(4):
                    kproj_chunk(het, chunk)

            def vproj(kt):
                """V rows for key tile kt (all heads) + bias + ones col."""
                for nch in range(2):
                    ps = pps.tile([P, Q] if not sc512 else [P, 512], F32, tag="sc", name="psv", bufs=2)
                    pss = ps[:, 0:512]
                    for dt in range(DT):
                        nc.tensor.matmul(
                            pss, x16[:, dt, ts(kt, P)],
                            wv16[:, dt, ts(nch, 512)],
                            start=(dt == 0),
                            stop=(zero_bias and dt == DT - 1))
                    if not zero_bias:
                        nc.tensor.matmul(
                            pss, ones[0:1, 0:P], bv_sb[0:1, ts(nch, 512)],
                            start=False, stop=True)
                    dst = vv[:, kt, nch * 8:(nch + 1) * 8, 0:64]
                    nc.vector.tensor_copy(
                        dst, pss.rearrange("p (h e) -> p h e", e=64))

            def qproj_chunk(hp, qT, qch):
                ps = pps.tile([P, Q] if not sc512 else [P, 512], F32, tag="sc", name="psq", bufs=2)
                pss = ps[:, 0:512]
                for dt in range(DT):
                    nc.tensor.matmul(
                        pss, wq16[:, dt, ts(hp, P)],
                        x16[:, dt, ts(qch, 512)],
                        start=(dt == 0), stop=(dt == DT - 1))
                nc.vector.tensor_tensor(
                    qT[:, ts(qch, 512)], pss,
                    bq_sb[:, hp:hp + 1].to_broadcast((P, 512)), ADD)

            def qproj(hp, qT):
                for qch in range(2):
                    qproj_chunk(hp, qT, qch)

            with (
                tc.tile_pool(name="wts", bufs=1) as wpool,
                tc.tile_pool(name="qt", bufs=2) as qpool,
                tc.tile_pool(name="exp", bufs=ebufs) as epool,
                tc.tile_pool(name="fin", bufs=1) as fpool,
            ):
                wk16 = wpool.tile([P, DT, HE], F16)
                wv16 = wpool.tile([P, DT, HE], F16)
                wq16 = wpool.tile([P, DT, HE], F16)
                with tc.tile_pool(name="wstg", bufs=2) as wstg:
                    for w16, wt in ((wk16, wkT_t), (wv16, wvT_t),
                                    (wq16, wqT_t)):
                        for dt in range(DT):
                            stg = wstg.tile([P, HE], F32, tag="wstg")
                            nc.sync.dma_start(stg[:], wt[:, dt, :])
                            nc.vector.tensor_copy(w16[:, dt, :], stg[:])

                if stage >= 2:
                    kproj(0)
                    qT_tiles = {0: qpool.tile([P, Q], F16, tag="qTn", name="qT0")}
                    qproj(0, qT_tiles[0])
                if stage == 2:
                    for hp in range(1, NHP):
                        kproj(hp)
                        qT_tiles[hp % 2] = qpool.tile([P, Q], F16, name="qTn")
                        qproj(hp, qT_tiles[hp % 2])
                    for kt in range(KT):
                        vproj(kt)
                def scores_exp(hp, qT, kt):
                    """Emit the 4 score matmuls (hh interleaved so adjacent
                    matmuls sit in different PE row groups and overlap) and
                    the exps for key tile kt. Returns the exp tiles."""
                    if sc512:
                        ets = [epool.tile([P, Q], F16, name=f"et{hh}")
                               for hh in range(2)]
                        for qch in range(2):
                            scs = []
                            for hh in range(2):
                                sc = pps.tile([P, 512], F32,
                                              tag=("sc" if hh == 0 else "sc2"),
                                              name=f"sc{hh}", bufs=2)
                                scs.append(sc)
                            for hh in range(2):
                                nc.tensor.matmul(
                                    scs[hh][:],
                                    kT16[hh * 64:(hh + 1) * 64, hp, ts(kt, P)],
                                    qT[hh * 64:(hh + 1) * 64, ts(qch, 512)],
                                    start=True, stop=True)
                            for hh in range(2):
                                nc.scalar.activation(
                                    ets[hh][:, ts(qch, 512)], scs[hh][:], Exp,
                                    bias=maskb[:, kt:kt + 1], scale=SCALE)
                        return ets
                    scs = []
                    for hh in range(2):
                        scs.append(pps.tile([P, Q], F32, tag="sc",
                                            name=f"sc{hh}", bufs=2))
                    for qch in range(2):
                        for hh in range(2):
                            nc.tensor.matmul(
                                scs[hh][:, ts(qch, 512)],
                                kT16[hh * 64:(hh + 1) * 64, hp, ts(kt, P)],
                                qT[hh * 64:(hh + 1) * 64, ts(qch, 512)],
                                start=True, stop=True)
                    ets = []
                    for hh in range(2):
                        et = epool.tile([P, Q], F16, name=f"et{hh}")
                        nc.scalar.activation(
                            et[:], scs[hh][:], Exp,
                            bias=maskb[:, kt:kt + 1], scale=SCALE)
                        ets.append(et)
                    return ets

                def av(hp, zt, kt, ets):
                    for hh in range(2):
                        h = hp * 2 + hh
                        for qch in range(2):
                            nc.tensor.matmul(
                                zt[hh * 2 + qch][:],
                                vv[:, kt, h, 0:65],
                                ets[hh][:, ts(qch, 512)],
                                start=(kt == 0), stop=(kt == KT - 1))

                if stage >= 3 and vpre:
                    for kt in range(4 if vstripe else KT):
                        vproj(kt)
                for hp in range(NHP if stage >= 3 else 0):
                    qT = qT_tiles.pop(hp)

                    zt = [pps.tile([65, 512], F32, tag=f"z{i}", name=f"z{i}")
                          for i in range(4)]
                    ets = scores_exp(hp, qT, 0)
                    for kt in range(KT):
                        if hp == 0 and not vpre:
                            vproj(kt)
                        if hp == 0 and vpre and vstripe and kt + 4 < KT:
                            vproj(kt + 4)
                        nxt = (scores_exp(hp, qT, kt + 1)
                               if kt + 1 < KT else None)
                        av(hp, zt, kt, ets)
                        ets = nxt
                        if striped and hp + 1 < NHP:
                            if kt in (4, 7, 10, 13):
                                kproj_chunk(hp + 1, (kt - 4) // 3)
                            elif kt == 5:
                                qT_tiles[hp + 1] = qpool.tile(
                                    [P, Q], F16, name="qTn")
                                qproj_chunk(hp + 1, qT_tiles[hp + 1], 0)
                            elif kt == 11:
                                qproj_chunk(hp + 1, qT_tiles[hp + 1], 1)
                    # next head-pair's projections run while ACT drains the
                    # last exps and DVE runs the normalize below
                    if not striped and hp + 1 < NHP:
                        kproj(hp + 1)
                        qT_tiles[hp + 1] = qpool.tile([P, Q], F16, name="qTn")
                        qproj(hp + 1, qT_tiles[hp + 1])

                    # --- normalize z and write mh^T (denominator in row 64)
                    # copy z to SBUF immediately to release the PSUM banks
                    zc = fpool.tile([65, 4, 512], F32, tag="zc")
                    for i in range(4):
                        nc.vector.tensor_copy(zc[:, i, :], zt[i][:])
                    nc.vector.reciprocal(zc[64:65, :, :], zc[64:65, :, :])
                    r16 = fpool.tile([65, 4, 512], F16, tag="r16")
                    with nc.allow_low_precision(reason="softmax recip f16"):
                        nc.vector.tensor_scalar_mul(
                            r16[64:65, :, :], zc[64:65, :, :], 256.0)
                    for hh in range(2):
                        for qch in range(2):
                            i = hh * 2 + qch
                            rep = pps.tile([64, 512], F32, tag=f"z{i}",
                                           name="rep")
                            nc.tensor.matmul(
                                rep[:], ones[64:65, 0:64],
                                r16[64:65, i, :], start=True, stop=True)
                            if hh == 0:
                                nc.vector.scalar_tensor_tensor(
                                    mh16[0:64, hp, ts(qch, 512)], zc[0:64, i, :],
                                    1.0 / 256.0, rep[:], MUL, MUL)
                            else:
                                tmp = fpool.tile([64, 512], F16, tag="tmp")
                                nc.vector.scalar_tensor_tensor(
                                    tmp[:], zc[0:64, i, :], 1.0 / 256.0, rep[:],
                                    MUL, MUL)
                                nc.sync.dma_start(
                                    mh16[64:128, hp, ts(qch, 512)], tmp[:])

            # ================= output projection ====================
            with (
                tc.tile_pool(name="wo", bufs=1) as wopool,
                tc.tile_pool(name="ost", bufs=3) as opool,
            ):
                wo16 = wopool.tile([P, NHP, D], F16)
                with tc.tile_pool(name="wostg", bufs=2) as wostg:
                    for dt in range(DT):
                        stg = wostg.tile([P, D], F32, tag="wostg")
                        nc.sync.dma_start(stg[:], woT_t[:, dt, :])
                        nc.vector.tensor_copy(wo16[:, dt, :], stg[:])
                for nch in range(2):
                    if stage < 4:
                        break
                    for qt in range(8):
                        ps = pps.tile([P, Q] if not sc512 else [P, 512], F32, tag="sc", name="pso", bufs=2)
                        pss = ps[:, 0:512]
                        for hp in range(NHP):
                            nc.tensor.matmul(
                                pss, mh16[:, hp, ts(qt, P)],
                                wo16[:, hp, ts(nch, 512)],
                                start=(hp == 0),
                                stop=(zero_bias and hp == NHP - 1))
                        if not zero_bias:
                            nc.tensor.matmul(
                                pss, ones[0:1, 0:P], bo_sb[0:1, ts(nch, 512)],
                                start=False, stop=True)
                        ot = opool.tile([P, 512], F32)
                        nc.vector.tensor_copy(ot[:], pss)
                        nc.sync.dma_start(out_t[:, qt, ts(nch, 512)], ot[:])

    nc.compile()
    return nc


def _kernel_dense(x, attention_mask, Wq, bq, Wk, bk, Wv, bv, Wo, bo):
    x = np.ascontiguousarray(np.asarray(x, dtype=np.float32))
    attention_mask = np.asarray(attention_mask, dtype=np.int32)
    B = x.shape[0]
    wqT = np.ascontiguousarray(np.asarray(Wq).transpose(2, 0, 1).reshape(D, HE))
    wkT = np.ascontiguousarray(np.asarray(Wk).transpose(2, 0, 1).reshape(D, HE))
    wvT = np.ascontiguousarray(np.asarray(Wv).transpose(2, 0, 1).reshape(D, HE))
    woT = np.ascontiguousarray(np.asarray(Wo).T.astype(np.float32))
    bqf = np.asarray(bq, dtype=np.float32).reshape(HE)
    bkf = np.asarray(bk, dtype=np.float32).reshape(HE)
    bvf = np.asarray(bv, dtype=np.float32).reshape(HE)
    bof = np.asarray(bo, dtype=np.float32).reshape(D)

    in_maps = []
    for c in range(N_CORES):
        b, qh = c // 2, c % 2
        qoff = qh * Q
        xTc = np.ascontiguousarray(np.roll(x[b].T, -qoff, axis=1))
        mk = np.ascontiguousarray(np.roll(attention_mask[b, 0], -qoff))
        in_maps.append({
            "xT": xTc, "wqT": wqT, "wkT": wkT, "wvT": wvT, "woT": woT,
            "mask": mk, "bq": bqf, "bk": bkf, "bv": bvf, "bo": bof,
        })

    zb = bool(np.all(bqf == 0) and np.all(bkf == 0) and np.all(bvf == 0)
              and np.all(bof == 0))
    key = ("dense", zb)
    if key not in _NC_CACHE:
        _NC_CACHE[key] = build_nc_dense(zero_bias=zb)
    nc = _NC_CACHE[key]
    res = bass_utils.run_bass_kernel_spmd(
        nc, in_maps, core_ids=list(range(N_CORES)))
    out = np.empty((B, S, D), dtype=np.float32)
    for c in range(N_CORES):
        b, qh = c // 2, c % 2
        out[b, qh * Q:(qh + 1) * Q, :] = res.results[c]["out"]
    return out
